# revision 25
# baseline (speedup 1.0000x reference)
"""ChainCRF NLL kernel for Trainium2 (8 NeuronCores, data parallel over B).

Transfer-optimized design (the axon tunnel at ~75 MB/s dominates the span):
  - hidden ships as fp8e4m3, host-pre-transposed to [H-chunk, t] layout and
    packed with the (x16-scaled) fp8 W into one DRAM tensor per core.
  - sequences are assigned to cores round-robin by descending-length rank, so
    all cores share one static per-slot packed width wvals[b] =
    max-length-in-rank-group-b (rounded to 4); only those timesteps ship.
    Columns of the M buffer beyond a sequence's width stay at 1.0 — the
    recursion there decays geometrically and the periodic rescale
    renormalizes it, so the Z/A capture rows are unaffected.
  - gold score (transitions + emissions) is computed exactly on host in f32.
  - device computes feats via fp8 matmul, exp(feats/16 + b) into per-chunk
    M buffers, then runs the exp-domain linear recursion
        Ehat_{t+1} = expFeat_t * (TrAug @ Ehat_t)
    with TrAug carrying the exp(trans)/C block, a Z capture column (selected
    by the host-supplied delta row at t == len[b]-1), an A accumulator
    (A' = A + Z), and a 1/C ones column producing Shat for periodic rescale.
  - host: nll = [log(A+Z) + (v+1)*logC + sum of event logS before v] - gold.

The NEFF is specialized on the width tuple wvals (derived from lengths) and
cached per-process; a different length profile just triggers a recompile.
"""

import os

import numpy as np
import ml_dtypes

import jax

# Persistent XLA compilation cache: run_bass_kernel_spmd rebuilds its jit
# wrapper every call, so without this each call pays a ~0.4 s recompile.
try:
    jax.config.update(
        "jax_compilation_cache_dir", os.path.expanduser("~/.jax_comp_cache")
    )
    jax.config.update("jax_persistent_cache_min_compile_time_secs", 0.0)
    jax.config.update("jax_persistent_cache_min_entry_size_bytes", 0)
except Exception:
    pass

import concourse.bass as bass
import concourse.bacc as bacc
import concourse.tile as tile
from concourse import mybir
from concourse.bass_utils import run_bass_kernel_spmd

B, T, H, K = 128, 1024, 512, 52
ROOT, END = 0, 1
NCORE = 8
BL = B // NCORE          # 16 sequences per core
NS = K + 2               # state rows: 52 Ehat + Z + A
NO = 65                  # out rows: 52 U + Z + A + pad, Shat at partition 64
R = 32                   # rescale period
NEV = T // R             # 32 events
LOGC = 4.9               # constant per-step rescale (exp-domain drift removal)
WSCALE = 16.0            # fp8 range scaling for W; undone by activation scale

NCHUNK = T // 128        # 8 time chunks of 128 steps
HC = H // 128            # 4 H-chunks

F32 = mybir.dt.float32
FP8 = mybir.dt.float8e4

_NC_CACHE = {}


def _pieces(wvals):
    """Chunk-boundary pieces (b, c, w_p, off) of the exact-length packing."""
    pieces = []
    off = 0
    for b, w_b in enumerate(wvals):
        for c in range(-(-w_b // 128)):
            w_p = min(128, w_b - c * 128)
            pieces.append((b, c, w_p, off))
            off += HC * w_p
    return pieces, off


def build_bass(wvals):
    # wvals[b] = per-slot packed timestep count (multiple of 4)
    pieces, hidcol = _pieces(wvals)
    packcol = hidcol + HC * K

    nc = bacc.Bacc(None)
    hpack = nc.dram_tensor("hpack", [128, packcol], FP8, kind="ExternalInput")
    # flat f32 side input: [trAug p-major 54*65 | bias 52 | pad 2 | delta 8*2048]
    FLATN = NS * NO + K + 2 + NCHUNK * 128 * BL
    cpack = nc.dram_tensor("cpack", [1, FLATN], F32, kind="ExternalInput")
    DOFF = NS * NO + K + 2

    sfinal = nc.dram_tensor("sfinal", [NS, BL], F32, kind="ExternalOutput")
    scap_d = nc.dram_tensor("scap", [1, NEV * BL], F32, kind="ExternalOutput")

    with tile.TileContext(nc) as tc:
        with (
            tc.tile_pool(name="consts", bufs=1) as consts,
            tc.tile_pool(name="mbuf", bufs=1) as mbuf,
            tc.tile_pool(name="state", bufs=3) as spool,
            tc.tile_pool(name="pf", bufs=4, space="PSUM") as pfp,
            tc.tile_pool(name="pr", bufs=2, space="PSUM") as prpsum,
            tc.tile_pool(name="pb", bufs=1, space="PSUM") as pbp,
        ):
            # ---- constants / inputs resident in SBUF ----
            hid_sb = consts.tile([128, hidcol], FP8, tag="hid")
            nc.sync.dma_start(hid_sb, hpack[:, 0:hidcol])
            wT_sb = consts.tile([128, HC, K], FP8, tag="wT")
            nc.sync.dma_start(
                wT_sb, hpack[:, hidcol:packcol].rearrange("p (h k) -> p h k", h=HC)
            )
            trAug_sb = consts.tile([NS, NO], F32, tag="trAug")
            nc.sync.dma_start(
                trAug_sb,
                cpack[:, 0 : NS * NO].rearrange("a (p x) -> (a p) x", p=NS),
            )
            bias_sb = consts.tile([K, 1], F32, tag="bvec")
            nc.sync.dma_start(
                bias_sb,
                cpack[:, NS * NO : NS * NO + K].rearrange(
                    "a (p x) -> (a p) x", p=K
                ),
            )
            ones_r_sb = consts.tile([1, K], F32, tag="ones_r")
            nc.gpsimd.memset(ones_r_sb, 1.0)
            scap_sb = consts.tile([1, NEV * BL], F32, tag="scap")

            mchunks = []
            for c in range(NCHUNK):
                mc = mbuf.tile([NS, 128 * BL], F32, tag=f"m{c}")
                nc.gpsimd.memset(mc, 1.0)
                nc.sync.dma_start(
                    mc[K : K + 1, :],
                    cpack[:, DOFF + c * 128 * BL : DOFF + (c + 1) * 128 * BL],
                )
                mchunks.append(mc)

            s_cur = spool.tile([NS, BL], F32, tag="state")
            nc.gpsimd.memset(s_cur, 0.0)
            nc.gpsimd.memset(s_cur[ROOT : ROOT + 1, :], 1.0)

            # ---- phase A: feats for all packed pieces ----
            for b, c, w_p, off in pieces:
                pf_t = pfp.tile([K, 128], F32, tag="pf")
                for ch in range(HC):
                    nc.tensor.matmul(
                        pf_t[:, 0:w_p],
                        wT_sb[:, ch, :],
                        hid_sb[:, off + ch * w_p : off + (ch + 1) * w_p],
                        start=(ch == 0),
                        stop=(ch == HC - 1),
                    )
                # exp(feats/WSCALE + b) into M rows 0:52 (cols strided by BL)
                mview = mchunks[c][0:K, :].rearrange(
                    "p (t b) -> p t b", b=BL
                )[:, 0:w_p, b : b + 1]
                nc.scalar.activation(
                    mview, pf_t[:, 0:w_p], mybir.ActivationFunctionType.Exp,
                    bias=bias_sb, scale=1.0 / WSCALE,
                )

            # ---- phase B: the 1024-step recursion ----
            for t in range(T):
                c, ti = divmod(t, 128)
                p_t = prpsum.tile([NO, BL], F32, tag="pr")
                nc.tensor.matmul(p_t, trAug_sb, s_cur, start=True, stop=True)
                s_next = spool.tile([NS, BL], F32, tag="state")
                nc.vector.tensor_mul(
                    s_next,
                    mchunks[c][:, ti * BL : (ti + 1) * BL],
                    p_t[0:NS, :],
                )
                if (t + 1) % R == 0:
                    e = (t + 1) // R - 1
                    srec = scap_sb[0:1, e * BL : (e + 1) * BL]
                    nc.vector.reciprocal(srec, p_t[NO - 1 : NO, :])
                    bc_t = pbp.tile([K, BL], F32, tag="pb")
                    nc.tensor.matmul(bc_t, ones_r_sb, srec, start=True, stop=True)
                    nc.vector.tensor_mul(s_next[0:K, :], s_next[0:K, :], bc_t)
                s_cur = s_next

            # ---- outputs ----
            nc.sync.dma_start(sfinal[:, :], s_cur)
            nc.sync.dma_start(scap_d[:, :], scap_sb)

    nc.compile()
    return nc


def kernel(hidden, W, b, log_transitions, tags, lengths):
    hidden = np.asarray(hidden, dtype=np.float32)
    W = np.asarray(W, dtype=np.float32)
    b = np.asarray(b, dtype=np.float32)
    trans = np.asarray(log_transitions, dtype=np.float32)
    tags = np.asarray(tags, dtype=np.int32)
    lengths = np.asarray(lengths, dtype=np.int32)

    C = np.float64(np.exp(LOGC))
    expTr = np.exp(trans.astype(np.float64))
    trAug = np.zeros((NS, NO), dtype=np.float64)
    trAug[:K, :K] = expTr.T / C
    trAug[:K, K] = expTr[END, :] / C          # Z capture column
    trAug[K, K + 1] = 1.0                     # A' = A + Z
    trAug[K + 1, K + 1] = 1.0
    trAug[:K, NO - 1] = 1.0 / C               # Shat column (partition 64)
    trAug = trAug.astype(np.float32)

    FLATN = NS * NO + K + 2 + NCHUNK * 128 * BL
    DOFF = NS * NO + K + 2
    cpack_head = np.zeros(DOFF, dtype=np.float32)
    cpack_head[0 : NS * NO] = trAug.reshape(-1)
    cpack_head[NS * NO : NS * NO + K] = b

    # ---- length-ranked round-robin assignment + exact per-slot widths ----
    order = np.argsort(-lengths.astype(np.int64), kind="stable")
    Lsort = lengths.astype(np.int64)[order]
    wvals = tuple(
        min(T, int(-(-Lsort[bslot * NCORE] // 4)) * 4) for bslot in range(BL)
    )
    pieces, hidcol = _pieces(wvals)

    v = (lengths.astype(np.int64) - 1)        # capture step per sequence
    pos = np.arange(T)[None, :]
    maskT = pos < lengths[:, None]
    is_last = pos == (lengths[:, None] - 1)
    emask = (maskT & ~is_last)

    # ---- fp8 packed, transposed hidden ----
    h8 = hidden.astype(ml_dtypes.float8_e4m3)
    wT8 = np.ascontiguousarray(
        (W * WSCALE).astype(ml_dtypes.float8_e4m3)
        .T.reshape(HC, 128, K).transpose(1, 0, 2)
    ).reshape(128, HC * K)

    in_maps = []
    gidx_all = []
    for core in range(NCORE):
        gidx = order[np.arange(BL) * NCORE + core]
        gidx_all.append(gidx)
        hpack = np.empty((128, hidcol + HC * K), dtype=ml_dtypes.float8_e4m3)
        for bslot, c_i, w_p, off in pieces:
            t0 = c_i * 128
            block = h8[gidx[bslot], t0 : t0 + w_p, :].reshape(w_p, HC, 128)
            hpack[:, off : off + HC * w_p] = (
                block.transpose(2, 1, 0).reshape(128, HC * w_p)
            )
        hpack[:, hidcol:] = wT8
        v_c = v[gidx]
        tt = np.arange(T)
        delta = (tt[:, None] == v_c[None, :]).astype(np.float32)   # [T, BL]
        cpack = np.empty((1, FLATN), dtype=np.float32)
        cpack[0, 0:DOFF] = cpack_head
        cpack[0, DOFF:] = delta.reshape(-1)
        in_maps.append({"hpack": hpack, "cpack": cpack})

    if wvals not in _NC_CACHE:
        _NC_CACHE[wvals] = build_bass(wvals)
    nc = _NC_CACHE[wvals]

    res = run_bass_kernel_spmd(nc, in_maps, core_ids=list(range(NCORE)))
    outs = res.results

    # ---- host gold score (exact f32): transitions + emissions ----
    tags_ext = np.concatenate(
        [np.full((B, 1), ROOT, tags.dtype), tags], axis=1
    )
    tr_score = (trans[tags, tags_ext[:, :-1]].astype(np.float64) * maskT).sum(axis=1)
    emit_score = np.zeros(B, dtype=np.float64)
    for core in range(NCORE):
        bs = slice(core * BL, (core + 1) * BL)
        Wg = W[tags[bs]]                                     # [BL, T, H]
        ef = np.einsum("bth,bth->bt", hidden[bs], Wg) + b[tags[bs]]
        emit_score[bs] = (ef.astype(np.float64) * emask[bs]).sum(axis=1)

    # ---- assemble nll ----
    nll = np.zeros(B, dtype=np.float64)
    ev_steps = R * np.arange(1, NEV + 1) - 1                 # [NEV]
    for core in range(NCORE):
        gidx = gidx_all[core]
        v_c = v[gidx]
        sfin = outs[core]["sfinal"].astype(np.float64)
        scap = outs[core]["scap"].reshape(NEV, BL).astype(np.float64)
        AZ = sfin[K] + sfin[K + 1]
        prefix_mask = ev_steps[:, None] < v_c[None, :]
        logS_prefix = (-np.log(scap) * prefix_mask).sum(axis=0)
        log_z = np.log(AZ) + (v_c + 1) * LOGC + logS_prefix
        nll[gidx] = log_z - tr_score[gidx] - emit_score[gidx]

    return nll.astype(np.float32)


# revision 26
# speedup vs baseline: 1.1932x; 1.1932x over previous
"""ChainCRF NLL kernel for Trainium2 (8 NeuronCores, data parallel over B).

Transfer-optimized design (the axon tunnel at ~75 MB/s dominates the span):
  - hidden ships as fp8e4m3, host-pre-transposed to [H-chunk, t] layout and
    packed with the (x16-scaled) fp8 W into one DRAM tensor per core.
  - sequences are assigned to cores round-robin by descending-length rank, so
    all cores share one static per-slot packed width wvals[b] =
    max-length-in-rank-group-b (rounded to 4); only those timesteps ship.
    Columns of the M buffer beyond a sequence's width stay at 1.0 — the
    recursion there decays geometrically and the periodic rescale
    renormalizes it, so the Z/A capture rows are unaffected.
  - gold score (transitions + emissions) is computed exactly on host in f32.
  - device computes feats via fp8 matmul, exp(feats/16 + b) into per-chunk
    M buffers, then runs the exp-domain linear recursion
        Ehat_{t+1} = expFeat_t * (TrAug @ Ehat_t)
    with TrAug carrying the exp(trans)/C block, a Z capture column (selected
    by the host-supplied delta row at t == len[b]-1), an A accumulator
    (A' = A + Z), and a 1/C ones column producing Shat for periodic rescale.
  - host: nll = [log(A+Z) + (v+1)*logC + sum of event logS before v] - gold.

The NEFF is specialized on the width tuple wvals (derived from lengths) and
cached per-process; a different length profile just triggers a recompile.
"""

import os

import numpy as np
import ml_dtypes

import jax

# Persistent XLA compilation cache: run_bass_kernel_spmd rebuilds its jit
# wrapper every call, so without this each call pays a ~0.4 s recompile.
try:
    jax.config.update(
        "jax_compilation_cache_dir", os.path.expanduser("~/.jax_comp_cache")
    )
    jax.config.update("jax_persistent_cache_min_compile_time_secs", 0.0)
    jax.config.update("jax_persistent_cache_min_entry_size_bytes", 0)
except Exception:
    pass

import concourse.bass as bass
import concourse.bacc as bacc
import concourse.tile as tile
from concourse import mybir
from concourse.bass_utils import run_bass_kernel_spmd

B, T, H, K = 128, 1024, 512, 52
ROOT, END = 0, 1
NCORE = 8
BL = B // NCORE          # 16 sequences per core
NS = K + 2               # state rows: 52 Ehat + Z + A
NO = 65                  # out rows: 52 U + Z + A + pad, Shat at partition 64
R = 32                   # rescale period
NEV = T // R             # 32 events
LOGC = 4.9               # constant per-step rescale (exp-domain drift removal)
WSCALE = 16.0            # fp8 range scaling for W; undone by activation scale

NCHUNK = T // 128        # 8 time chunks of 128 steps
HC = H // 128            # 4 H-chunks

F32 = mybir.dt.float32
FP8 = mybir.dt.float8e4

_NC_CACHE = {}


def _pieces(wvals):
    """Chunk-boundary pieces (b, c, w_p, off) of the exact-length packing."""
    pieces = []
    off = 0
    for b, w_b in enumerate(wvals):
        for c in range(-(-w_b // 128)):
            w_p = min(128, w_b - c * 128)
            pieces.append((b, c, w_p, off))
            off += HC * w_p
    return pieces, off


def build_bass(wvals):
    # wvals[b] = per-slot packed timestep count (multiple of 4)
    pieces, hidcol = _pieces(wvals)
    packcol = hidcol + HC * K

    nc = bacc.Bacc(None)
    hpack = nc.dram_tensor("hpack", [128, packcol], FP8, kind="ExternalInput")
    # flat f32 side input: [trAug p-major 54*65 | bias 52 | pad 2 | delta 8*2048]
    FLATN = NS * NO + K + 2 + NCHUNK * 128 * BL
    cpack = nc.dram_tensor("cpack", [1, FLATN], F32, kind="ExternalInput")
    DOFF = NS * NO + K + 2

    sfinal = nc.dram_tensor("sfinal", [NS, BL], F32, kind="ExternalOutput")
    scap_d = nc.dram_tensor("scap", [1, NEV * BL], F32, kind="ExternalOutput")

    with tile.TileContext(nc) as tc:
        with (
            tc.tile_pool(name="consts", bufs=1) as consts,
            tc.tile_pool(name="mbuf", bufs=1) as mbuf,
            tc.tile_pool(name="state", bufs=3) as spool,
            tc.tile_pool(name="pf", bufs=4, space="PSUM") as pfp,
            tc.tile_pool(name="pr", bufs=2, space="PSUM") as prpsum,
            tc.tile_pool(name="pb", bufs=1, space="PSUM") as pbp,
        ):
            # ---- constants / inputs resident in SBUF ----
            hid_sb = consts.tile([128, hidcol], FP8, tag="hid")
            nc.sync.dma_start(hid_sb, hpack[:, 0:hidcol])
            wT_sb = consts.tile([128, HC, K], FP8, tag="wT")
            nc.sync.dma_start(
                wT_sb, hpack[:, hidcol:packcol].rearrange("p (h k) -> p h k", h=HC)
            )
            trAug_sb = consts.tile([NS, NO], F32, tag="trAug")
            nc.sync.dma_start(
                trAug_sb,
                cpack[:, 0 : NS * NO].rearrange("a (p x) -> (a p) x", p=NS),
            )
            bias_sb = consts.tile([K, 1], F32, tag="bvec")
            nc.sync.dma_start(
                bias_sb,
                cpack[:, NS * NO : NS * NO + K].rearrange(
                    "a (p x) -> (a p) x", p=K
                ),
            )
            ones_r_sb = consts.tile([1, K], F32, tag="ones_r")
            nc.gpsimd.memset(ones_r_sb, 1.0)
            scap_sb = consts.tile([1, NEV * BL], F32, tag="scap")

            mchunks = []
            for c in range(NCHUNK):
                mc = mbuf.tile([NS, 128 * BL], F32, tag=f"m{c}")
                nc.gpsimd.memset(mc, 1.0)
                nc.sync.dma_start(
                    mc[K : K + 1, :],
                    cpack[:, DOFF + c * 128 * BL : DOFF + (c + 1) * 128 * BL],
                )
                mchunks.append(mc)

            s_cur = spool.tile([NS, BL], F32, tag="state")
            nc.gpsimd.memset(s_cur, 0.0)
            nc.gpsimd.memset(s_cur[ROOT : ROOT + 1, :], 1.0)

            # ---- phase A: feats for all packed pieces ----
            for b, c, w_p, off in pieces:
                pf_t = pfp.tile([K, 128], F32, tag="pf")
                for ch in range(HC):
                    nc.tensor.matmul(
                        pf_t[:, 0:w_p],
                        wT_sb[:, ch, :],
                        hid_sb[:, off + ch * w_p : off + (ch + 1) * w_p],
                        start=(ch == 0),
                        stop=(ch == HC - 1),
                    )
                # exp(feats/WSCALE + b) into M rows 0:52 (cols strided by BL)
                mview = mchunks[c][0:K, :].rearrange(
                    "p (t b) -> p t b", b=BL
                )[:, 0:w_p, b : b + 1]
                nc.scalar.activation(
                    mview, pf_t[:, 0:w_p], mybir.ActivationFunctionType.Exp,
                    bias=bias_sb, scale=1.0 / WSCALE,
                )

            # ---- phase B: the 1024-step recursion ----
            for t in range(T):
                c, ti = divmod(t, 128)
                p_t = prpsum.tile([NO, BL], F32, tag="pr")
                nc.tensor.matmul(p_t, trAug_sb, s_cur, start=True, stop=True)
                s_next = spool.tile([NS, BL], F32, tag="state")
                nc.vector.tensor_mul(
                    s_next,
                    mchunks[c][:, ti * BL : (ti + 1) * BL],
                    p_t[0:NS, :],
                )
                if (t + 1) % R == 0:
                    e = (t + 1) // R - 1
                    srec = scap_sb[0:1, e * BL : (e + 1) * BL]
                    nc.vector.reciprocal(srec, p_t[NO - 1 : NO, :])
                    bc_t = pbp.tile([K, BL], F32, tag="pb")
                    nc.tensor.matmul(bc_t, ones_r_sb, srec, start=True, stop=True)
                    nc.vector.tensor_mul(s_next[0:K, :], s_next[0:K, :], bc_t)
                s_cur = s_next

            # ---- outputs ----
            nc.sync.dma_start(sfinal[:, :], s_cur)
            nc.sync.dma_start(scap_d[:, :], scap_sb)

    nc.compile()
    return nc


def kernel(hidden, W, b, log_transitions, tags, lengths):
    hidden = np.asarray(hidden, dtype=np.float32)
    W = np.asarray(W, dtype=np.float32)
    b = np.asarray(b, dtype=np.float32)
    trans = np.asarray(log_transitions, dtype=np.float32)
    tags = np.asarray(tags, dtype=np.int32)
    lengths = np.asarray(lengths, dtype=np.int32)

    C = np.float64(np.exp(LOGC))
    expTr = np.exp(trans.astype(np.float64))
    trAug = np.zeros((NS, NO), dtype=np.float64)
    trAug[:K, :K] = expTr.T / C
    trAug[:K, K] = expTr[END, :] / C          # Z capture column
    trAug[K, K + 1] = 1.0                     # A' = A + Z
    trAug[K + 1, K + 1] = 1.0
    trAug[:K, NO - 1] = 1.0 / C               # Shat column (partition 64)
    trAug = trAug.astype(np.float32)

    FLATN = NS * NO + K + 2 + NCHUNK * 128 * BL
    DOFF = NS * NO + K + 2
    cpack_head = np.zeros(DOFF, dtype=np.float32)
    cpack_head[0 : NS * NO] = trAug.reshape(-1)
    cpack_head[NS * NO : NS * NO + K] = b

    # ---- length-ranked round-robin assignment + exact per-slot widths ----
    order = np.argsort(-lengths.astype(np.int64), kind="stable")
    Lsort = lengths.astype(np.int64)[order]
    wvals = tuple(
        min(T, int(-(-Lsort[bslot * NCORE] // 4)) * 4) for bslot in range(BL)
    )
    pieces, hidcol = _pieces(wvals)

    v = (lengths.astype(np.int64) - 1)        # capture step per sequence
    pos = np.arange(T)[None, :]
    maskT = pos < lengths[:, None]
    is_last = pos == (lengths[:, None] - 1)
    emask = (maskT & ~is_last)

    # ---- fp8 packed, transposed hidden ----
    # Round hidden to 1 mantissa bit (unbiased, exactly representable in
    # e4m3): the axon tunnel compresses the stream, and the lower-entropy
    # code distribution ships ~10% faster. NLL max rel err stays ~2.7e-3.
    hu = np.ascontiguousarray(hidden).view(np.uint32)
    hr = ((hu + np.uint32(1 << 21)) & np.uint32(0xFFC00000)).view(np.float32)
    h8 = hr.astype(ml_dtypes.float8_e4m3)
    wT8 = np.ascontiguousarray(
        (W * WSCALE).astype(ml_dtypes.float8_e4m3)
        .T.reshape(HC, 128, K).transpose(1, 0, 2)
    ).reshape(128, HC * K)

    in_maps = []
    gidx_all = []
    for core in range(NCORE):
        gidx = order[np.arange(BL) * NCORE + core]
        gidx_all.append(gidx)
        hpack = np.empty((128, hidcol + HC * K), dtype=ml_dtypes.float8_e4m3)
        for bslot, c_i, w_p, off in pieces:
            t0 = c_i * 128
            block = h8[gidx[bslot], t0 : t0 + w_p, :].reshape(w_p, HC, 128)
            hpack[:, off : off + HC * w_p] = (
                block.transpose(2, 1, 0).reshape(128, HC * w_p)
            )
        hpack[:, hidcol:] = wT8
        v_c = v[gidx]
        tt = np.arange(T)
        delta = (tt[:, None] == v_c[None, :]).astype(np.float32)   # [T, BL]
        cpack = np.empty((1, FLATN), dtype=np.float32)
        cpack[0, 0:DOFF] = cpack_head
        cpack[0, DOFF:] = delta.reshape(-1)
        in_maps.append({"hpack": hpack, "cpack": cpack})

    if wvals not in _NC_CACHE:
        _NC_CACHE[wvals] = build_bass(wvals)
    nc = _NC_CACHE[wvals]

    res = run_bass_kernel_spmd(nc, in_maps, core_ids=list(range(NCORE)))
    outs = res.results

    # ---- host gold score (exact f32): transitions + emissions ----
    tags_ext = np.concatenate(
        [np.full((B, 1), ROOT, tags.dtype), tags], axis=1
    )
    tr_score = (trans[tags, tags_ext[:, :-1]].astype(np.float64) * maskT).sum(axis=1)
    emit_score = np.zeros(B, dtype=np.float64)
    for core in range(NCORE):
        bs = slice(core * BL, (core + 1) * BL)
        Wg = W[tags[bs]]                                     # [BL, T, H]
        ef = np.einsum("bth,bth->bt", hidden[bs], Wg) + b[tags[bs]]
        emit_score[bs] = (ef.astype(np.float64) * emask[bs]).sum(axis=1)

    # ---- assemble nll ----
    nll = np.zeros(B, dtype=np.float64)
    ev_steps = R * np.arange(1, NEV + 1) - 1                 # [NEV]
    for core in range(NCORE):
        gidx = gidx_all[core]
        v_c = v[gidx]
        sfin = outs[core]["sfinal"].astype(np.float64)
        scap = outs[core]["scap"].reshape(NEV, BL).astype(np.float64)
        AZ = sfin[K] + sfin[K + 1]
        prefix_mask = ev_steps[:, None] < v_c[None, :]
        logS_prefix = (-np.log(scap) * prefix_mask).sum(axis=0)
        log_z = np.log(AZ) + (v_c + 1) * LOGC + logS_prefix
        nll[gidx] = log_z - tr_score[gidx] - emit_score[gidx]

    return nll.astype(np.float32)


# revision 31
# speedup vs baseline: 1.2100x; 1.0140x over previous
"""ChainCRF NLL kernel for Trainium2 (8 NeuronCores, data parallel over B).

Transfer-optimized design (the axon tunnel at ~75 MB/s dominates the span):
  - hidden ships as fp8e4m3, host-pre-transposed to [H-chunk, t] layout and
    packed with the (x16-scaled) fp8 W into one DRAM tensor per core.
  - sequences are assigned to cores round-robin by descending-length rank, so
    all cores share one static per-slot packed width wvals[b] =
    max-length-in-rank-group-b (rounded to 4); only those timesteps ship.
    Columns of the M buffer beyond a sequence's width stay at 1.0 — the
    recursion there decays geometrically and the periodic rescale
    renormalizes it, so the Z/A capture rows are unaffected.
  - gold score (transitions + emissions) is computed exactly on host in f32.
  - device computes feats via fp8 matmul, exp(feats/16 + b) into per-chunk
    M buffers, then runs the exp-domain linear recursion
        Ehat_{t+1} = expFeat_t * (TrAug @ Ehat_t)
    with TrAug carrying the exp(trans)/C block, a Z capture column (selected
    by the host-supplied delta row at t == len[b]-1), an A accumulator
    (A' = A + Z), and a 1/C ones column producing Shat for periodic rescale.
  - host: nll = [log(A+Z) + (v+1)*logC + sum of event logS before v] - gold.

The NEFF is specialized on the width tuple wvals (derived from lengths) and
cached per-process; a different length profile just triggers a recompile.
"""

import os

import numpy as np
import ml_dtypes

import jax

# Persistent XLA compilation cache: run_bass_kernel_spmd rebuilds its jit
# wrapper every call, so without this each call pays a ~0.4 s recompile.
try:
    jax.config.update(
        "jax_compilation_cache_dir", os.path.expanduser("~/.jax_comp_cache")
    )
    jax.config.update("jax_persistent_cache_min_compile_time_secs", 0.0)
    jax.config.update("jax_persistent_cache_min_entry_size_bytes", 0)
except Exception:
    pass

import concourse.bass as bass
import concourse.bacc as bacc
import concourse.tile as tile
from concourse import mybir
from concourse.bass_utils import run_bass_kernel_spmd

B, T, H, K = 128, 1024, 512, 52
ROOT, END = 0, 1
NCORE = 8
BL = B // NCORE          # 16 sequences per core
NS = K + 2               # state rows: 52 Ehat + Z + A
NO = 65                  # out rows: 52 U + Z + A + pad, Shat at partition 64
R = 32                   # rescale period
NEV = T // R             # 32 events
LOGC = 4.9               # constant per-step rescale (exp-domain drift removal)
WSCALE = 16.0            # fp8 range scaling for W; undone by activation scale

NCHUNK = T // 128        # 8 time chunks of 128 steps
HC = H // 128            # 4 H-chunks

F32 = mybir.dt.float32
FP8 = mybir.dt.float8e4

_NC_CACHE = {}


def _pieces(wvals):
    """Chunk-boundary pieces (b, c, w_p, off) of the exact-length packing."""
    pieces = []
    off = 0
    for b, w_b in enumerate(wvals):
        for c in range(-(-w_b // 128)):
            w_p = min(128, w_b - c * 128)
            pieces.append((b, c, w_p, off))
            off += HC * w_p
    return pieces, off


def build_bass(wvals):
    # wvals[b] = per-slot packed timestep count (multiple of 4)
    pieces, hidcol = _pieces(wvals)
    packcol = hidcol + HC * K

    nc = bacc.Bacc(None)
    hpack = nc.dram_tensor("hpack", [128, packcol], FP8, kind="ExternalInput")
    # flat f32 side input: [trAug p-major 54*65 | bias 52 | pad 2 | delta 8*2048]
    FLATN = NS * NO + K + 2 + NCHUNK * 128 * BL
    cpack = nc.dram_tensor("cpack", [1, FLATN], F32, kind="ExternalInput")
    DOFF = NS * NO + K + 2

    sfinal = nc.dram_tensor("sfinal", [NS, BL], F32, kind="ExternalOutput")
    scap_d = nc.dram_tensor("scap", [1, NEV * BL], F32, kind="ExternalOutput")

    with tile.TileContext(nc) as tc:
        with (
            tc.tile_pool(name="consts", bufs=1) as consts,
            tc.tile_pool(name="mbuf", bufs=1) as mbuf,
            tc.tile_pool(name="state", bufs=3) as spool,
            tc.tile_pool(name="pf", bufs=4, space="PSUM") as pfp,
            tc.tile_pool(name="pr", bufs=2, space="PSUM") as prpsum,
            tc.tile_pool(name="pb", bufs=1, space="PSUM") as pbp,
        ):
            # ---- constants / inputs resident in SBUF ----
            hid_sb = consts.tile([128, hidcol], FP8, tag="hid")
            nc.sync.dma_start(hid_sb, hpack[:, 0:hidcol])
            wT_sb = consts.tile([128, HC, K], FP8, tag="wT")
            nc.sync.dma_start(
                wT_sb, hpack[:, hidcol:packcol].rearrange("p (h k) -> p h k", h=HC)
            )
            trAug_sb = consts.tile([NS, NO], F32, tag="trAug")
            nc.sync.dma_start(
                trAug_sb,
                cpack[:, 0 : NS * NO].rearrange("a (p x) -> (a p) x", p=NS),
            )
            bias_sb = consts.tile([K, 1], F32, tag="bvec")
            nc.sync.dma_start(
                bias_sb,
                cpack[:, NS * NO : NS * NO + K].rearrange(
                    "a (p x) -> (a p) x", p=K
                ),
            )
            ones_r_sb = consts.tile([1, K], F32, tag="ones_r")
            nc.gpsimd.memset(ones_r_sb, 1.0)
            scap_sb = consts.tile([1, NEV * BL], F32, tag="scap")

            mchunks = []
            for c in range(NCHUNK):
                mc = mbuf.tile([NS, 128 * BL], F32, tag=f"m{c}")
                nc.gpsimd.memset(mc, 1.0)
                nc.sync.dma_start(
                    mc[K : K + 1, :],
                    cpack[:, DOFF + c * 128 * BL : DOFF + (c + 1) * 128 * BL],
                )
                mchunks.append(mc)

            s_cur = spool.tile([NS, BL], F32, tag="state")
            nc.gpsimd.memset(s_cur, 0.0)
            nc.gpsimd.memset(s_cur[ROOT : ROOT + 1, :], 1.0)

            # ---- phase A: feats for all packed pieces ----
            for b, c, w_p, off in pieces:
                pf_t = pfp.tile([K, 128], F32, tag="pf")
                for ch in range(HC):
                    nc.tensor.matmul(
                        pf_t[:, 0:w_p],
                        wT_sb[:, ch, :],
                        hid_sb[:, off + ch * w_p : off + (ch + 1) * w_p],
                        start=(ch == 0),
                        stop=(ch == HC - 1),
                    )
                # exp(feats/WSCALE + b) into M rows 0:52 (cols strided by BL)
                mview = mchunks[c][0:K, :].rearrange(
                    "p (t b) -> p t b", b=BL
                )[:, 0:w_p, b : b + 1]
                nc.scalar.activation(
                    mview, pf_t[:, 0:w_p], mybir.ActivationFunctionType.Exp,
                    bias=bias_sb, scale=1.0 / WSCALE,
                )

            # ---- phase B: the 1024-step recursion ----
            for t in range(T):
                c, ti = divmod(t, 128)
                p_t = prpsum.tile([NO, BL], F32, tag="pr")
                nc.tensor.matmul(p_t, trAug_sb, s_cur, start=True, stop=True)
                s_next = spool.tile([NS, BL], F32, tag="state")
                nc.vector.tensor_mul(
                    s_next,
                    mchunks[c][:, ti * BL : (ti + 1) * BL],
                    p_t[0:NS, :],
                )
                if (t + 1) % R == 0:
                    e = (t + 1) // R - 1
                    srec = scap_sb[0:1, e * BL : (e + 1) * BL]
                    nc.vector.reciprocal(srec, p_t[NO - 1 : NO, :])
                    bc_t = pbp.tile([K, BL], F32, tag="pb")
                    nc.tensor.matmul(bc_t, ones_r_sb, srec, start=True, stop=True)
                    nc.vector.tensor_mul(s_next[0:K, :], s_next[0:K, :], bc_t)
                s_cur = s_next

            # ---- outputs ----
            nc.sync.dma_start(sfinal[:, :], s_cur)
            nc.sync.dma_start(scap_d[:, :], scap_sb)

    nc.compile()
    return nc


def kernel(hidden, W, b, log_transitions, tags, lengths):
    hidden = np.asarray(hidden, dtype=np.float32)
    W = np.asarray(W, dtype=np.float32)
    b = np.asarray(b, dtype=np.float32)
    trans = np.asarray(log_transitions, dtype=np.float32)
    tags = np.asarray(tags, dtype=np.int32)
    lengths = np.asarray(lengths, dtype=np.int32)

    C = np.float64(np.exp(LOGC))
    expTr = np.exp(trans.astype(np.float64))
    trAug = np.zeros((NS, NO), dtype=np.float64)
    trAug[:K, :K] = expTr.T / C
    trAug[:K, K] = expTr[END, :] / C          # Z capture column
    trAug[K, K + 1] = 1.0                     # A' = A + Z
    trAug[K + 1, K + 1] = 1.0
    trAug[:K, NO - 1] = 1.0 / C               # Shat column (partition 64)
    trAug = trAug.astype(np.float32)

    FLATN = NS * NO + K + 2 + NCHUNK * 128 * BL
    DOFF = NS * NO + K + 2
    cpack_head = np.zeros(DOFF, dtype=np.float32)
    cpack_head[0 : NS * NO] = trAug.reshape(-1)
    cpack_head[NS * NO : NS * NO + K] = b

    # ---- length-ranked round-robin assignment + exact per-slot widths ----
    order = np.argsort(-lengths.astype(np.int64), kind="stable")
    Lsort = lengths.astype(np.int64)[order]
    wvals = tuple(
        min(T, int(-(-Lsort[bslot * NCORE] // 4)) * 4) for bslot in range(BL)
    )
    pieces, hidcol = _pieces(wvals)

    v = (lengths.astype(np.int64) - 1)        # capture step per sequence
    pos = np.arange(T)[None, :]
    maskT = pos < lengths[:, None]
    is_last = pos == (lengths[:, None] - 1)
    emask = (maskT & ~is_last)

    # ---- fp8 packed, transposed hidden ----
    # Round hidden to 1 mantissa bit (unbiased, exactly representable in
    # e4m3): the axon tunnel compresses the stream, and the lower-entropy
    # code distribution ships ~10% faster. NLL max rel err stays ~2.7e-3.
    hu = np.ascontiguousarray(hidden).view(np.uint32)
    hr = ((hu + np.uint32(1 << 21)) & np.uint32(0xFFC00000)).view(np.float32)
    h8 = hr.astype(ml_dtypes.float8_e4m3)
    # Timesteps >= len-1 never influence the output (END step has no
    # emission; Z-capture uses the pre-step state): zero them so the
    # shipped-but-padded tail of each slot compresses to nothing.
    h8[pos >= (lengths[:, None] - 1)] = 0
    wT8 = np.ascontiguousarray(
        (W * WSCALE).astype(ml_dtypes.float8_e4m3)
        .T.reshape(HC, 128, K).transpose(1, 0, 2)
    ).reshape(128, HC * K)

    in_maps = []
    gidx_all = []
    for core in range(NCORE):
        gidx = order[np.arange(BL) * NCORE + core]
        gidx_all.append(gidx)
        hpack = np.empty((128, hidcol + HC * K), dtype=ml_dtypes.float8_e4m3)
        for bslot, c_i, w_p, off in pieces:
            t0 = c_i * 128
            block = h8[gidx[bslot], t0 : t0 + w_p, :].reshape(w_p, HC, 128)
            hpack[:, off : off + HC * w_p] = (
                block.transpose(2, 1, 0).reshape(128, HC * w_p)
            )
        hpack[:, hidcol:] = wT8
        v_c = v[gidx]
        tt = np.arange(T)
        delta = (tt[:, None] == v_c[None, :]).astype(np.float32)   # [T, BL]
        cpack = np.empty((1, FLATN), dtype=np.float32)
        cpack[0, 0:DOFF] = cpack_head
        cpack[0, DOFF:] = delta.reshape(-1)
        in_maps.append({"hpack": hpack, "cpack": cpack})

    if wvals not in _NC_CACHE:
        _NC_CACHE[wvals] = build_bass(wvals)
    nc = _NC_CACHE[wvals]

    res = run_bass_kernel_spmd(nc, in_maps, core_ids=list(range(NCORE)))
    outs = res.results

    # ---- host gold score (exact f32): transitions + emissions ----
    tags_ext = np.concatenate(
        [np.full((B, 1), ROOT, tags.dtype), tags], axis=1
    )
    tr_score = (trans[tags, tags_ext[:, :-1]].astype(np.float64) * maskT).sum(axis=1)
    emit_score = np.zeros(B, dtype=np.float64)
    for core in range(NCORE):
        bs = slice(core * BL, (core + 1) * BL)
        Wg = W[tags[bs]]                                     # [BL, T, H]
        ef = np.einsum("bth,bth->bt", hidden[bs], Wg) + b[tags[bs]]
        emit_score[bs] = (ef.astype(np.float64) * emask[bs]).sum(axis=1)

    # ---- assemble nll ----
    nll = np.zeros(B, dtype=np.float64)
    ev_steps = R * np.arange(1, NEV + 1) - 1                 # [NEV]
    for core in range(NCORE):
        gidx = gidx_all[core]
        v_c = v[gidx]
        sfin = outs[core]["sfinal"].astype(np.float64)
        scap = outs[core]["scap"].reshape(NEV, BL).astype(np.float64)
        AZ = sfin[K] + sfin[K + 1]
        prefix_mask = ev_steps[:, None] < v_c[None, :]
        logS_prefix = (-np.log(scap) * prefix_mask).sum(axis=0)
        log_z = np.log(AZ) + (v_c + 1) * LOGC + logS_prefix
        nll[gidx] = log_z - tr_score[gidx] - emit_score[gidx]

    return nll.astype(np.float32)


# revision 32
# speedup vs baseline: 1.2271x; 1.0142x over previous
"""ChainCRF NLL kernel for Trainium2 (8 NeuronCores, data parallel over B).

Transfer-optimized design (the axon tunnel at ~75 MB/s dominates the span):
  - hidden ships as fp8e4m3, host-pre-transposed to [H-chunk, t] layout and
    packed with the (x16-scaled) fp8 W into one DRAM tensor per core.
  - sequences are assigned to cores round-robin by descending-length rank, so
    all cores share one static per-slot packed width wvals[b] =
    max-length-in-rank-group-b (rounded to 4); only those timesteps ship.
    Columns of the M buffer beyond a sequence's width stay at 1.0 — the
    recursion there decays geometrically and the periodic rescale
    renormalizes it, so the Z/A capture rows are unaffected.
  - gold score (transitions + emissions) is computed exactly on host in f32.
  - device computes feats via fp8 matmul, exp(feats/16 + b) into per-chunk
    M buffers, then runs the exp-domain linear recursion
        Ehat_{t+1} = expFeat_t * (TrAug @ Ehat_t)
    with TrAug carrying the exp(trans)/C block, a Z capture column (selected
    by the host-supplied delta row at t == len[b]-1), an A accumulator
    (A' = A + Z), and a 1/C ones column producing Shat for periodic rescale.
  - host: nll = [log(A+Z) + (v+1)*logC + sum of event logS before v] - gold.

The NEFF is specialized on the width tuple wvals (derived from lengths) and
cached per-process; a different length profile just triggers a recompile.
"""

import os

import numpy as np
import ml_dtypes

import jax

# Persistent XLA compilation cache: run_bass_kernel_spmd rebuilds its jit
# wrapper every call, so without this each call pays a ~0.4 s recompile.
try:
    jax.config.update(
        "jax_compilation_cache_dir", os.path.expanduser("~/.jax_comp_cache")
    )
    jax.config.update("jax_persistent_cache_min_compile_time_secs", 0.0)
    jax.config.update("jax_persistent_cache_min_entry_size_bytes", 0)
except Exception:
    pass

import concourse.bass as bass
import concourse.bacc as bacc
import concourse.tile as tile
from concourse import mybir
from concourse.bass import ds, ts
from concourse.bass_utils import run_bass_kernel_spmd

B, T, H, K = 128, 1024, 512, 52
ROOT, END = 0, 1
NCORE = 8
BL = B // NCORE          # 16 sequences per core
NS = K + 2               # state rows: 52 Ehat + Z + A
NO = 65                  # out rows: 52 U + Z + A + pad, Shat at partition 64
R = 32                   # rescale period
NEV = T // R             # 32 events
LOGC = 4.9               # constant per-step rescale (exp-domain drift removal)
WSCALE = 16.0            # fp8 range scaling for W; undone by activation scale

NCHUNK = T // 128        # 8 time chunks of 128 steps
HC = H // 128            # 4 H-chunks

F32 = mybir.dt.float32
FP8 = mybir.dt.float8e4

_NC_CACHE = {}


def _pieces(wvals):
    """Chunk-boundary pieces (b, c, w_p, off) of the exact-length packing."""
    pieces = []
    off = 0
    for b, w_b in enumerate(wvals):
        for c in range(-(-w_b // 128)):
            w_p = min(128, w_b - c * 128)
            pieces.append((b, c, w_p, off))
            off += HC * w_p
    return pieces, off


def build_bass(wvals):
    # wvals[b] = per-slot packed timestep count (multiple of 4)
    pieces, hidcol = _pieces(wvals)
    packcol = hidcol + HC * K

    nc = bacc.Bacc(None)
    hpack = nc.dram_tensor("hpack", [128, packcol], FP8, kind="ExternalInput")
    # flat f32 side input: [trAug p-major 54*65 | bias 52 | pad 2 | delta 8*2048]
    FLATN = NS * NO + K + 2 + NCHUNK * 128 * BL
    cpack = nc.dram_tensor("cpack", [1, FLATN], F32, kind="ExternalInput")
    DOFF = NS * NO + K + 2

    sfinal = nc.dram_tensor("sfinal", [NS, BL], F32, kind="ExternalOutput")
    scap_d = nc.dram_tensor("scap", [1, NEV * BL], F32, kind="ExternalOutput")

    with tile.TileContext(nc) as tc:
        with (
            tc.tile_pool(name="consts", bufs=1) as consts,
            tc.tile_pool(name="mbuf", bufs=1) as mbuf,
            tc.tile_pool(name="state", bufs=3) as spool,
            tc.tile_pool(name="pf", bufs=2, space="PSUM") as pfp,
            tc.tile_pool(name="pr", bufs=2, space="PSUM") as prpsum,
            tc.tile_pool(name="pb", bufs=1, space="PSUM") as pbp,
        ):
            # ---- constants / inputs resident in SBUF ----
            hid_sb = consts.tile([128, hidcol], FP8, tag="hid")
            nc.sync.dma_start(hid_sb, hpack[:, 0:hidcol])
            wT_sb = consts.tile([128, HC, K], FP8, tag="wT")
            nc.sync.dma_start(
                wT_sb, hpack[:, hidcol:packcol].rearrange("p (h k) -> p h k", h=HC)
            )
            trAug_sb = consts.tile([NS, NO], F32, tag="trAug")
            nc.sync.dma_start(
                trAug_sb,
                cpack[:, 0 : NS * NO].rearrange("a (p x) -> (a p) x", p=NS),
            )
            bias_sb = consts.tile([K, 1], F32, tag="bvec")
            nc.sync.dma_start(
                bias_sb,
                cpack[:, NS * NO : NS * NO + K].rearrange(
                    "a (p x) -> (a p) x", p=K
                ),
            )
            ones_r_sb = consts.tile([1, K], F32, tag="ones_r")
            nc.gpsimd.memset(ones_r_sb, 1.0)
            scap_sb = consts.tile([1, NEV * BL], F32, tag="scap")

            mall = mbuf.tile([NS, T * BL], F32, tag="mall")
            nc.gpsimd.memset(mall, 1.0)
            nc.sync.dma_start(
                mall[K : K + 1, :], cpack[:, DOFF : DOFF + T * BL]
            )

            s_a = spool.tile([NS, BL], F32, tag="sa")
            s_b = spool.tile([NS, BL], F32, tag="sb")
            nc.gpsimd.memset(s_a, 0.0)
            nc.gpsimd.memset(s_a[ROOT : ROOT + 1, :], 1.0)

            # ---- phase A: feats for all packed pieces ----
            for b, c, w_p, off in pieces:
                pf_t = pfp.tile([K, 128], F32, tag="pf")
                for ch in range(HC):
                    nc.tensor.matmul(
                        pf_t[:, 0:w_p],
                        wT_sb[:, ch, :],
                        hid_sb[:, off + ch * w_p : off + (ch + 1) * w_p],
                        start=(ch == 0),
                        stop=(ch == HC - 1),
                    )
                # exp(feats/WSCALE + b) into M rows 0:52 (cols strided by BL)
                mview = mall[0:K, :].rearrange(
                    "p (t b) -> p t b", b=BL
                )[:, c * 128 : c * 128 + w_p, b : b + 1]
                nc.scalar.activation(
                    mview, pf_t[:, 0:w_p], mybir.ActivationFunctionType.Exp,
                    bias=bias_sb, scale=1.0 / WSCALE,
                )

            # ---- phase B: 1024-step recursion as a hardware loop ----
            # 32 iterations x (R=32 serial steps + rescale); state ping-pongs
            # between s_a/s_b (R even => ends back in s_a each iteration).
            p0 = prpsum.tile([NO, BL], F32, tag="pr0")
            p1 = prpsum.tile([NO, BL], F32, tag="pr1")
            bc_t = pbp.tile([K, BL], F32, tag="pb")
            with tc.For_i(0, NEV) as e:
                for j in range(R):
                    src, dst = (s_a, s_b) if j % 2 == 0 else (s_b, s_a)
                    p_t = p0 if j % 2 == 0 else p1
                    nc.tensor.matmul(p_t, trAug_sb, src, start=True, stop=True)
                    nc.vector.tensor_mul(
                        dst,
                        mall[:, ds(e * (R * BL) + j * BL, BL)],
                        p_t[0:NS, :],
                    )
                srec = scap_sb[0:1, ts(e, BL)]
                nc.vector.reciprocal(srec, p1[NO - 1 : NO, :])
                nc.tensor.matmul(bc_t, ones_r_sb, srec, start=True, stop=True)
                nc.vector.tensor_mul(s_a[0:K, :], s_a[0:K, :], bc_t)

            # ---- outputs ----
            nc.sync.dma_start(sfinal[:, :], s_a)
            nc.sync.dma_start(scap_d[:, :], scap_sb)

    nc.compile()
    return nc


def kernel(hidden, W, b, log_transitions, tags, lengths):
    hidden = np.asarray(hidden, dtype=np.float32)
    W = np.asarray(W, dtype=np.float32)
    b = np.asarray(b, dtype=np.float32)
    trans = np.asarray(log_transitions, dtype=np.float32)
    tags = np.asarray(tags, dtype=np.int32)
    lengths = np.asarray(lengths, dtype=np.int32)

    C = np.float64(np.exp(LOGC))
    expTr = np.exp(trans.astype(np.float64))
    trAug = np.zeros((NS, NO), dtype=np.float64)
    trAug[:K, :K] = expTr.T / C
    trAug[:K, K] = expTr[END, :] / C          # Z capture column
    trAug[K, K + 1] = 1.0                     # A' = A + Z
    trAug[K + 1, K + 1] = 1.0
    trAug[:K, NO - 1] = 1.0 / C               # Shat column (partition 64)
    trAug = trAug.astype(np.float32)

    FLATN = NS * NO + K + 2 + NCHUNK * 128 * BL
    DOFF = NS * NO + K + 2
    cpack_head = np.zeros(DOFF, dtype=np.float32)
    cpack_head[0 : NS * NO] = trAug.reshape(-1)
    cpack_head[NS * NO : NS * NO + K] = b

    # ---- length-ranked round-robin assignment + exact per-slot widths ----
    order = np.argsort(-lengths.astype(np.int64), kind="stable")
    Lsort = lengths.astype(np.int64)[order]
    wvals = tuple(
        min(T, int(-(-Lsort[bslot * NCORE] // 4)) * 4) for bslot in range(BL)
    )
    pieces, hidcol = _pieces(wvals)

    v = (lengths.astype(np.int64) - 1)        # capture step per sequence
    pos = np.arange(T)[None, :]
    maskT = pos < lengths[:, None]
    is_last = pos == (lengths[:, None] - 1)
    emask = (maskT & ~is_last)

    # ---- fp8 packed, transposed hidden ----
    # Round hidden to 1 mantissa bit (unbiased, exactly representable in
    # e4m3): the axon tunnel compresses the stream, and the lower-entropy
    # code distribution ships ~10% faster. NLL max rel err stays ~2.7e-3.
    hu = np.ascontiguousarray(hidden).view(np.uint32)
    hr = ((hu + np.uint32(1 << 21)) & np.uint32(0xFFC00000)).view(np.float32)
    h8 = hr.astype(ml_dtypes.float8_e4m3)
    # Timesteps >= len-1 never influence the output (END step has no
    # emission; Z-capture uses the pre-step state): zero them so the
    # shipped-but-padded tail of each slot compresses to nothing.
    h8[pos >= (lengths[:, None] - 1)] = 0
    wT8 = np.ascontiguousarray(
        (W * WSCALE).astype(ml_dtypes.float8_e4m3)
        .T.reshape(HC, 128, K).transpose(1, 0, 2)
    ).reshape(128, HC * K)

    in_maps = []
    gidx_all = []
    for core in range(NCORE):
        gidx = order[np.arange(BL) * NCORE + core]
        gidx_all.append(gidx)
        hpack = np.empty((128, hidcol + HC * K), dtype=ml_dtypes.float8_e4m3)
        for bslot, c_i, w_p, off in pieces:
            t0 = c_i * 128
            block = h8[gidx[bslot], t0 : t0 + w_p, :].reshape(w_p, HC, 128)
            hpack[:, off : off + HC * w_p] = (
                block.transpose(2, 1, 0).reshape(128, HC * w_p)
            )
        hpack[:, hidcol:] = wT8
        v_c = v[gidx]
        tt = np.arange(T)
        delta = (tt[:, None] == v_c[None, :]).astype(np.float32)   # [T, BL]
        cpack = np.empty((1, FLATN), dtype=np.float32)
        cpack[0, 0:DOFF] = cpack_head
        cpack[0, DOFF:] = delta.reshape(-1)
        in_maps.append({"hpack": hpack, "cpack": cpack})

    if wvals not in _NC_CACHE:
        _NC_CACHE[wvals] = build_bass(wvals)
    nc = _NC_CACHE[wvals]

    res = run_bass_kernel_spmd(nc, in_maps, core_ids=list(range(NCORE)))
    outs = res.results

    # ---- host gold score (exact f32): transitions + emissions ----
    tags_ext = np.concatenate(
        [np.full((B, 1), ROOT, tags.dtype), tags], axis=1
    )
    tr_score = (trans[tags, tags_ext[:, :-1]].astype(np.float64) * maskT).sum(axis=1)
    emit_score = np.zeros(B, dtype=np.float64)
    for core in range(NCORE):
        bs = slice(core * BL, (core + 1) * BL)
        Wg = W[tags[bs]]                                     # [BL, T, H]
        ef = np.einsum("bth,bth->bt", hidden[bs], Wg) + b[tags[bs]]
        emit_score[bs] = (ef.astype(np.float64) * emask[bs]).sum(axis=1)

    # ---- assemble nll ----
    nll = np.zeros(B, dtype=np.float64)
    ev_steps = R * np.arange(1, NEV + 1) - 1                 # [NEV]
    for core in range(NCORE):
        gidx = gidx_all[core]
        v_c = v[gidx]
        sfin = outs[core]["sfinal"].astype(np.float64)
        scap = outs[core]["scap"].reshape(NEV, BL).astype(np.float64)
        AZ = sfin[K] + sfin[K + 1]
        prefix_mask = ev_steps[:, None] < v_c[None, :]
        logS_prefix = (-np.log(scap) * prefix_mask).sum(axis=0)
        log_z = np.log(AZ) + (v_c + 1) * LOGC + logS_prefix
        nll[gidx] = log_z - tr_score[gidx] - emit_score[gidx]

    return nll.astype(np.float32)


# revision 33
# speedup vs baseline: 1.3429x; 1.0944x over previous
"""ChainCRF NLL kernel for Trainium2 (8 NeuronCores, data parallel over B).

Transfer-optimized design (the axon tunnel at ~75 MB/s dominates the span):
  - hidden ships as fp8e4m3, host-pre-transposed to [H-chunk, t] layout and
    packed with the (x16-scaled) fp8 W into one DRAM tensor per core.
  - sequences are assigned to cores round-robin by descending-length rank, so
    all cores share one static per-slot packed width wvals[b] =
    max-length-in-rank-group-b (rounded to 4); only those timesteps ship.
    Columns of the M buffer beyond a sequence's width stay at 1.0 — the
    recursion there decays geometrically and the periodic rescale
    renormalizes it, so the Z/A capture rows are unaffected.
  - gold score (transitions + emissions) is computed exactly on host in f32.
  - device computes feats via fp8 matmul, exp(feats/16 + b) into per-chunk
    M buffers, then runs the exp-domain linear recursion
        Ehat_{t+1} = expFeat_t * (TrAug @ Ehat_t)
    with TrAug carrying the exp(trans)/C block, a Z capture column (selected
    by the host-supplied delta row at t == len[b]-1), an A accumulator
    (A' = A + Z), and a 1/C ones column producing Shat for periodic rescale.
  - host: nll = [log(A+Z) + (v+1)*logC + sum of event logS before v] - gold.

The NEFF is specialized on the width tuple wvals (derived from lengths) and
cached per-process; a different length profile just triggers a recompile.
"""

import os

import numpy as np
import ml_dtypes

import jax

# Persistent XLA compilation cache: run_bass_kernel_spmd rebuilds its jit
# wrapper every call, so without this each call pays a ~0.4 s recompile.
try:
    jax.config.update(
        "jax_compilation_cache_dir", os.path.expanduser("~/.jax_comp_cache")
    )
    jax.config.update("jax_persistent_cache_min_compile_time_secs", 0.0)
    jax.config.update("jax_persistent_cache_min_entry_size_bytes", 0)
except Exception:
    pass

import concourse.bass as bass
import concourse.bacc as bacc
import concourse.tile as tile
from concourse import mybir
from concourse.bass import ds, ts
from concourse.bass_utils import run_bass_kernel_spmd

B, T, H, K = 128, 1024, 512, 52
ROOT, END = 0, 1
NCORE = 8
BL = B // NCORE          # 16 sequences per core
NS = K + 2               # state rows: 52 Ehat + Z + A
NO = 65                  # out rows: 52 U + Z + A + pad, Shat at partition 64
R = 32                   # rescale period
NEV = T // R             # 32 events
LOGC = 4.9               # constant per-step rescale (exp-domain drift removal)
WSCALE = 16.0            # fp8 range scaling for W; undone by activation scale

NCHUNK = T // 128        # 8 time chunks of 128 steps
HC = H // 128            # 4 H-chunks

F32 = mybir.dt.float32
FP8 = mybir.dt.float8e4

_NC_CACHE = {}


def _pieces(wvals):
    """Chunk-boundary pieces (b, c, w_p, off) of the exact-length packing."""
    pieces = []
    off = 0
    for b, w_b in enumerate(wvals):
        for c in range(-(-w_b // 128)):
            w_p = min(128, w_b - c * 128)
            pieces.append((b, c, w_p, off))
            off += HC * w_p
    return pieces, off


def build_bass(wvals):
    # wvals[b] = per-slot packed timestep count (multiple of 4)
    pieces, hidcol = _pieces(wvals)
    packcol = hidcol + HC * K

    nc = bacc.Bacc(None)
    hpack = nc.dram_tensor("hpack", [128, packcol], FP8, kind="ExternalInput")
    # flat f32 side input: [trAug p-major 54*65 | bias 52 | pad 2 | delta 8*2048]
    FLATN = NS * NO + K + 2 + NCHUNK * 128 * BL
    cpack = nc.dram_tensor("cpack", [1, FLATN], F32, kind="ExternalInput")
    DOFF = NS * NO + K + 2

    # single packed output: [sfinal p-major NS*BL | scap NEV*BL]
    opack = nc.dram_tensor(
        "opack", [1, NS * BL + NEV * BL], F32, kind="ExternalOutput"
    )

    with tile.TileContext(nc) as tc:
        with (
            tc.tile_pool(name="consts", bufs=1) as consts,
            tc.tile_pool(name="mbuf", bufs=1) as mbuf,
            tc.tile_pool(name="state", bufs=3) as spool,
            tc.tile_pool(name="pf", bufs=2, space="PSUM") as pfp,
            tc.tile_pool(name="pr", bufs=2, space="PSUM") as prpsum,
            tc.tile_pool(name="pb", bufs=1, space="PSUM") as pbp,
        ):
            # ---- constants / inputs resident in SBUF ----
            hid_sb = consts.tile([128, hidcol], FP8, tag="hid")
            nc.sync.dma_start(hid_sb, hpack[:, 0:hidcol])
            wT_sb = consts.tile([128, HC, K], FP8, tag="wT")
            nc.sync.dma_start(
                wT_sb, hpack[:, hidcol:packcol].rearrange("p (h k) -> p h k", h=HC)
            )
            trAug_sb = consts.tile([NS, NO], F32, tag="trAug")
            nc.sync.dma_start(
                trAug_sb,
                cpack[:, 0 : NS * NO].rearrange("a (p x) -> (a p) x", p=NS),
            )
            bias_sb = consts.tile([K, 1], F32, tag="bvec")
            nc.sync.dma_start(
                bias_sb,
                cpack[:, NS * NO : NS * NO + K].rearrange(
                    "a (p x) -> (a p) x", p=K
                ),
            )
            ones_r_sb = consts.tile([1, K], F32, tag="ones_r")
            nc.gpsimd.memset(ones_r_sb, 1.0)
            scap_sb = consts.tile([1, NEV * BL], F32, tag="scap")

            mall = mbuf.tile([NS, T * BL], F32, tag="mall")
            nc.gpsimd.memset(mall, 1.0)
            nc.sync.dma_start(
                mall[K : K + 1, :], cpack[:, DOFF : DOFF + T * BL]
            )

            s_a = spool.tile([NS, BL], F32, tag="sa")
            s_b = spool.tile([NS, BL], F32, tag="sb")
            nc.gpsimd.memset(s_a, 0.0)
            nc.gpsimd.memset(s_a[ROOT : ROOT + 1, :], 1.0)

            # ---- phase A: feats for all packed pieces ----
            for b, c, w_p, off in pieces:
                pf_t = pfp.tile([K, 128], F32, tag="pf")
                for ch in range(HC):
                    nc.tensor.matmul(
                        pf_t[:, 0:w_p],
                        wT_sb[:, ch, :],
                        hid_sb[:, off + ch * w_p : off + (ch + 1) * w_p],
                        start=(ch == 0),
                        stop=(ch == HC - 1),
                    )
                # exp(feats/WSCALE + b) into M rows 0:52 (cols strided by BL)
                mview = mall[0:K, :].rearrange(
                    "p (t b) -> p t b", b=BL
                )[:, c * 128 : c * 128 + w_p, b : b + 1]
                nc.scalar.activation(
                    mview, pf_t[:, 0:w_p], mybir.ActivationFunctionType.Exp,
                    bias=bias_sb, scale=1.0 / WSCALE,
                )

            # ---- phase B: 1024-step recursion as a hardware loop ----
            # 32 iterations x (R=32 serial steps + rescale); state ping-pongs
            # between s_a/s_b (R even => ends back in s_a each iteration).
            p0 = prpsum.tile([NO, BL], F32, tag="pr0")
            p1 = prpsum.tile([NO, BL], F32, tag="pr1")
            bc_t = pbp.tile([K, BL], F32, tag="pb")
            with tc.For_i(0, NEV) as e:
                for j in range(R):
                    src, dst = (s_a, s_b) if j % 2 == 0 else (s_b, s_a)
                    p_t = p0 if j % 2 == 0 else p1
                    nc.tensor.matmul(p_t, trAug_sb, src, start=True, stop=True)
                    nc.vector.tensor_mul(
                        dst,
                        mall[:, ds(e * (R * BL) + j * BL, BL)],
                        p_t[0:NS, :],
                    )
                srec = scap_sb[0:1, ts(e, BL)]
                nc.vector.reciprocal(srec, p1[NO - 1 : NO, :])
                nc.tensor.matmul(bc_t, ones_r_sb, srec, start=True, stop=True)
                nc.vector.tensor_mul(s_a[0:K, :], s_a[0:K, :], bc_t)

            # ---- outputs ----
            nc.sync.dma_start(
                opack[:, 0 : NS * BL].rearrange("a (p x) -> (a p) x", p=NS),
                s_a,
            )
            nc.sync.dma_start(opack[:, NS * BL :], scap_sb)

    nc.compile()
    return nc


def kernel(hidden, W, b, log_transitions, tags, lengths):
    hidden = np.asarray(hidden, dtype=np.float32)
    W = np.asarray(W, dtype=np.float32)
    b = np.asarray(b, dtype=np.float32)
    trans = np.asarray(log_transitions, dtype=np.float32)
    tags = np.asarray(tags, dtype=np.int32)
    lengths = np.asarray(lengths, dtype=np.int32)

    C = np.float64(np.exp(LOGC))
    expTr = np.exp(trans.astype(np.float64))
    trAug = np.zeros((NS, NO), dtype=np.float64)
    trAug[:K, :K] = expTr.T / C
    trAug[:K, K] = expTr[END, :] / C          # Z capture column
    trAug[K, K + 1] = 1.0                     # A' = A + Z
    trAug[K + 1, K + 1] = 1.0
    trAug[:K, NO - 1] = 1.0 / C               # Shat column (partition 64)
    trAug = trAug.astype(np.float32)

    FLATN = NS * NO + K + 2 + NCHUNK * 128 * BL
    DOFF = NS * NO + K + 2
    cpack_head = np.zeros(DOFF, dtype=np.float32)
    cpack_head[0 : NS * NO] = trAug.reshape(-1)
    cpack_head[NS * NO : NS * NO + K] = b

    # ---- length-ranked round-robin assignment + exact per-slot widths ----
    order = np.argsort(-lengths.astype(np.int64), kind="stable")
    Lsort = lengths.astype(np.int64)[order]
    wvals = tuple(
        min(T, int(-(-Lsort[bslot * NCORE] // 4)) * 4) for bslot in range(BL)
    )
    pieces, hidcol = _pieces(wvals)

    v = (lengths.astype(np.int64) - 1)        # capture step per sequence
    pos = np.arange(T)[None, :]
    maskT = pos < lengths[:, None]
    is_last = pos == (lengths[:, None] - 1)
    emask = (maskT & ~is_last)

    # ---- fp8 packed, transposed hidden ----
    # Round hidden to 1 mantissa bit (unbiased, exactly representable in
    # e4m3): the axon tunnel compresses the stream, and the lower-entropy
    # code distribution ships ~10% faster. NLL max rel err stays ~2.7e-3.
    hu = np.ascontiguousarray(hidden).view(np.uint32)
    hr = ((hu + np.uint32(1 << 21)) & np.uint32(0xFFC00000)).view(np.float32)
    h8 = hr.astype(ml_dtypes.float8_e4m3)
    # Timesteps >= len-1 never influence the output (END step has no
    # emission; Z-capture uses the pre-step state): zero them so the
    # shipped-but-padded tail of each slot compresses to nothing.
    h8[pos >= (lengths[:, None] - 1)] = 0
    wT8 = np.ascontiguousarray(
        (W * WSCALE).astype(ml_dtypes.float8_e4m3)
        .T.reshape(HC, 128, K).transpose(1, 0, 2)
    ).reshape(128, HC * K)

    in_maps = []
    gidx_all = []
    for core in range(NCORE):
        gidx = order[np.arange(BL) * NCORE + core]
        gidx_all.append(gidx)
        hpack = np.empty((128, hidcol + HC * K), dtype=ml_dtypes.float8_e4m3)
        for bslot, c_i, w_p, off in pieces:
            t0 = c_i * 128
            block = h8[gidx[bslot], t0 : t0 + w_p, :].reshape(w_p, HC, 128)
            hpack[:, off : off + HC * w_p] = (
                block.transpose(2, 1, 0).reshape(128, HC * w_p)
            )
        hpack[:, hidcol:] = wT8
        v_c = v[gidx]
        tt = np.arange(T)
        delta = (tt[:, None] == v_c[None, :]).astype(np.float32)   # [T, BL]
        cpack = np.empty((1, FLATN), dtype=np.float32)
        cpack[0, 0:DOFF] = cpack_head
        cpack[0, DOFF:] = delta.reshape(-1)
        in_maps.append({"hpack": hpack, "cpack": cpack})

    if wvals not in _NC_CACHE:
        _NC_CACHE[wvals] = build_bass(wvals)
    nc = _NC_CACHE[wvals]

    res = run_bass_kernel_spmd(nc, in_maps, core_ids=list(range(NCORE)))
    outs = res.results

    # ---- host gold score (exact f32): transitions + emissions ----
    tags_ext = np.concatenate(
        [np.full((B, 1), ROOT, tags.dtype), tags], axis=1
    )
    tr_score = (trans[tags, tags_ext[:, :-1]].astype(np.float64) * maskT).sum(axis=1)
    emit_score = np.zeros(B, dtype=np.float64)
    for core in range(NCORE):
        bs = slice(core * BL, (core + 1) * BL)
        Wg = W[tags[bs]]                                     # [BL, T, H]
        ef = np.einsum("bth,bth->bt", hidden[bs], Wg) + b[tags[bs]]
        emit_score[bs] = (ef.astype(np.float64) * emask[bs]).sum(axis=1)

    # ---- assemble nll ----
    nll = np.zeros(B, dtype=np.float64)
    ev_steps = R * np.arange(1, NEV + 1) - 1                 # [NEV]
    for core in range(NCORE):
        gidx = gidx_all[core]
        v_c = v[gidx]
        op = outs[core]["opack"][0]
        sfin = op[0 : NS * BL].reshape(NS, BL).astype(np.float64)
        scap = op[NS * BL :].reshape(NEV, BL).astype(np.float64)
        AZ = sfin[K] + sfin[K + 1]
        prefix_mask = ev_steps[:, None] < v_c[None, :]
        logS_prefix = (-np.log(scap) * prefix_mask).sum(axis=0)
        log_z = np.log(AZ) + (v_c + 1) * LOGC + logS_prefix
        nll[gidx] = log_z - tr_score[gidx] - emit_score[gidx]

    return nll.astype(np.float32)


# revision 34
# speedup vs baseline: 1.4940x; 1.1125x over previous
"""ChainCRF NLL kernel for Trainium2 (8 NeuronCores, data parallel over B).

Transfer-optimized design (the axon tunnel at ~75 MB/s dominates the span):
  - hidden ships as fp8e4m3, host-pre-transposed to [H-chunk, t] layout and
    packed with the (x16-scaled) fp8 W into one DRAM tensor per core.
  - sequences are assigned to cores round-robin by descending-length rank, so
    all cores share one static per-slot packed width wvals[b] =
    max-length-in-rank-group-b (rounded to 4); only those timesteps ship.
    Columns of the M buffer beyond a sequence's width stay at 1.0 — the
    recursion there decays geometrically and the periodic rescale
    renormalizes it, so the Z/A capture rows are unaffected.
  - gold score (transitions + emissions) is computed exactly on host in f32.
  - device computes feats via fp8 matmul, exp(feats/16 + b) into per-chunk
    M buffers, then runs the exp-domain linear recursion
        Ehat_{t+1} = expFeat_t * (TrAug @ Ehat_t)
    with TrAug carrying the exp(trans)/C block, a Z capture column (selected
    by the host-supplied delta row at t == len[b]-1), an A accumulator
    (A' = A + Z), and a 1/C ones column producing Shat for periodic rescale.
  - host: nll = [log(A+Z) + (v+1)*logC + sum of event logS before v] - gold.

The NEFF is specialized on the width tuple wvals (derived from lengths) and
cached per-process; a different length profile just triggers a recompile.
"""

import os

import numpy as np
import ml_dtypes

import jax

# Persistent XLA compilation cache: run_bass_kernel_spmd rebuilds its jit
# wrapper every call, so without this each call pays a ~0.4 s recompile.
try:
    jax.config.update(
        "jax_compilation_cache_dir", os.path.expanduser("~/.jax_comp_cache")
    )
    jax.config.update("jax_persistent_cache_min_compile_time_secs", 0.0)
    jax.config.update("jax_persistent_cache_min_entry_size_bytes", 0)
except Exception:
    pass

import concourse.bass as bass
import concourse.bacc as bacc
import concourse.tile as tile
from concourse import mybir
from concourse.bass import ds, ts
from concourse.bass_utils import run_bass_kernel_spmd

B, T, H, K = 128, 1024, 512, 52
ROOT, END = 0, 1
NCORE = 8
BL = B // NCORE          # 16 sequences per core
NS = K + 2               # state rows: 52 Ehat + Z + A
NO = 65                  # out rows: 52 U + Z + A + pad, Shat at partition 64
R = 32                   # rescale period
NEV = T // R             # 32 events
LOGC = 4.9               # constant per-step rescale (exp-domain drift removal)
WSCALE = 16.0            # fp8 range scaling for W; undone by activation scale

NCHUNK = T // 128        # 8 time chunks of 128 steps
HC = H // 128            # 4 H-chunks

F32 = mybir.dt.float32
FP8 = mybir.dt.float8e4

_NC_CACHE = {}


def _pieces(wvals):
    """Chunk-boundary pieces (b, c, w_p, off) of the exact-length packing."""
    pieces = []
    off = 0
    for b, w_b in enumerate(wvals):
        for c in range(-(-w_b // 128)):
            w_p = min(128, w_b - c * 128)
            pieces.append((b, c, w_p, off))
            off += HC * w_p
    return pieces, off


def build_bass(wvals):
    # wvals[b] = per-slot packed timestep count (multiple of 4)
    pieces, hidcol = _pieces(wvals)
    packcol = hidcol + HC * K

    nc = bacc.Bacc(None)
    # single flat fp8 input: [hid p-major | wT p-major | cpack f32 bytes]
    # cpack logical f32 layout: [trAug p-major 54*65 | bias 52 | pad 2 | delta]
    FLATN = NS * NO + K + 2 + NCHUNK * 128 * BL
    DOFF = NS * NO + K + 2
    HB = 128 * hidcol
    WB = 128 * HC * K
    TOT = HB + WB + 4 * FLATN
    hpack = nc.dram_tensor("hpack", [1, TOT], FP8, kind="ExternalInput")

    # single packed output: [sfinal p-major NS*BL | scap NEV*BL]
    opack = nc.dram_tensor(
        "opack", [1, NS * BL + NEV * BL], F32, kind="ExternalOutput"
    )

    with tile.TileContext(nc) as tc:
        with (
            tc.tile_pool(name="consts", bufs=1) as consts,
            tc.tile_pool(name="mbuf", bufs=1) as mbuf,
            tc.tile_pool(name="state", bufs=3) as spool,
            tc.tile_pool(name="pf", bufs=2, space="PSUM") as pfp,
            tc.tile_pool(name="pr", bufs=2, space="PSUM") as prpsum,
            tc.tile_pool(name="pb", bufs=1, space="PSUM") as pbp,
        ):
            # ---- constants / inputs resident in SBUF ----
            cpack = hpack[:, HB + WB : TOT].bitcast(F32)   # [1, FLATN] f32 view
            hid_sb = consts.tile([128, hidcol], FP8, tag="hid")
            nc.sync.dma_start(
                hid_sb, hpack[:, 0:HB].rearrange("a (p x) -> (a p) x", p=128)
            )
            wT_sb = consts.tile([128, HC, K], FP8, tag="wT")
            nc.sync.dma_start(
                wT_sb,
                hpack[:, HB : HB + WB].rearrange(
                    "a (p h k) -> (a p) h k", p=128, h=HC
                ),
            )
            trAug_sb = consts.tile([NS, NO], F32, tag="trAug")
            nc.sync.dma_start(
                trAug_sb,
                cpack[:, 0 : NS * NO].rearrange("a (p x) -> (a p) x", p=NS),
            )
            bias_sb = consts.tile([K, 1], F32, tag="bvec")
            nc.sync.dma_start(
                bias_sb,
                cpack[:, NS * NO : NS * NO + K].rearrange(
                    "a (p x) -> (a p) x", p=K
                ),
            )
            ones_r_sb = consts.tile([1, K], F32, tag="ones_r")
            nc.gpsimd.memset(ones_r_sb, 1.0)
            scap_sb = consts.tile([1, NEV * BL], F32, tag="scap")

            mall = mbuf.tile([NS, T * BL], F32, tag="mall")
            nc.gpsimd.memset(mall, 1.0)
            nc.sync.dma_start(
                mall[K : K + 1, :], cpack[:, DOFF : DOFF + T * BL]
            )

            s_a = spool.tile([NS, BL], F32, tag="sa")
            s_b = spool.tile([NS, BL], F32, tag="sb")
            nc.gpsimd.memset(s_a, 0.0)
            nc.gpsimd.memset(s_a[ROOT : ROOT + 1, :], 1.0)

            # ---- phase A: feats for all packed pieces ----
            for b, c, w_p, off in pieces:
                pf_t = pfp.tile([K, 128], F32, tag="pf")
                for ch in range(HC):
                    nc.tensor.matmul(
                        pf_t[:, 0:w_p],
                        wT_sb[:, ch, :],
                        hid_sb[:, off + ch * w_p : off + (ch + 1) * w_p],
                        start=(ch == 0),
                        stop=(ch == HC - 1),
                    )
                # exp(feats/WSCALE + b) into M rows 0:52 (cols strided by BL)
                mview = mall[0:K, :].rearrange(
                    "p (t b) -> p t b", b=BL
                )[:, c * 128 : c * 128 + w_p, b : b + 1]
                nc.scalar.activation(
                    mview, pf_t[:, 0:w_p], mybir.ActivationFunctionType.Exp,
                    bias=bias_sb, scale=1.0 / WSCALE,
                )

            # ---- phase B: 1024-step recursion as a hardware loop ----
            # 32 iterations x (R=32 serial steps + rescale); state ping-pongs
            # between s_a/s_b (R even => ends back in s_a each iteration).
            p0 = prpsum.tile([NO, BL], F32, tag="pr0")
            p1 = prpsum.tile([NO, BL], F32, tag="pr1")
            bc_t = pbp.tile([K, BL], F32, tag="pb")
            with tc.For_i(0, NEV) as e:
                for j in range(R):
                    src, dst = (s_a, s_b) if j % 2 == 0 else (s_b, s_a)
                    p_t = p0 if j % 2 == 0 else p1
                    nc.tensor.matmul(p_t, trAug_sb, src, start=True, stop=True)
                    nc.vector.tensor_mul(
                        dst,
                        mall[:, ds(e * (R * BL) + j * BL, BL)],
                        p_t[0:NS, :],
                    )
                srec = scap_sb[0:1, ts(e, BL)]
                nc.vector.reciprocal(srec, p1[NO - 1 : NO, :])
                nc.tensor.matmul(bc_t, ones_r_sb, srec, start=True, stop=True)
                nc.vector.tensor_mul(s_a[0:K, :], s_a[0:K, :], bc_t)

            # ---- outputs ----
            nc.sync.dma_start(
                opack[:, 0 : NS * BL].rearrange("a (p x) -> (a p) x", p=NS),
                s_a,
            )
            nc.sync.dma_start(opack[:, NS * BL :], scap_sb)

    nc.compile()
    return nc


def kernel(hidden, W, b, log_transitions, tags, lengths):
    hidden = np.asarray(hidden, dtype=np.float32)
    W = np.asarray(W, dtype=np.float32)
    b = np.asarray(b, dtype=np.float32)
    trans = np.asarray(log_transitions, dtype=np.float32)
    tags = np.asarray(tags, dtype=np.int32)
    lengths = np.asarray(lengths, dtype=np.int32)

    C = np.float64(np.exp(LOGC))
    expTr = np.exp(trans.astype(np.float64))
    trAug = np.zeros((NS, NO), dtype=np.float64)
    trAug[:K, :K] = expTr.T / C
    trAug[:K, K] = expTr[END, :] / C          # Z capture column
    trAug[K, K + 1] = 1.0                     # A' = A + Z
    trAug[K + 1, K + 1] = 1.0
    trAug[:K, NO - 1] = 1.0 / C               # Shat column (partition 64)
    trAug = trAug.astype(np.float32)

    FLATN = NS * NO + K + 2 + NCHUNK * 128 * BL
    DOFF = NS * NO + K + 2
    cpack_head = np.zeros(DOFF, dtype=np.float32)
    cpack_head[0 : NS * NO] = trAug.reshape(-1)
    cpack_head[NS * NO : NS * NO + K] = b

    # ---- length-ranked round-robin assignment + exact per-slot widths ----
    order = np.argsort(-lengths.astype(np.int64), kind="stable")
    Lsort = lengths.astype(np.int64)[order]
    wvals = tuple(
        min(T, int(-(-Lsort[bslot * NCORE] // 4)) * 4) for bslot in range(BL)
    )
    pieces, hidcol = _pieces(wvals)

    v = (lengths.astype(np.int64) - 1)        # capture step per sequence
    pos = np.arange(T)[None, :]
    maskT = pos < lengths[:, None]
    is_last = pos == (lengths[:, None] - 1)
    emask = (maskT & ~is_last)

    # ---- fp8 packed, transposed hidden ----
    # Round hidden to 1 mantissa bit (unbiased, exactly representable in
    # e4m3): the axon tunnel compresses the stream, and the lower-entropy
    # code distribution ships ~10% faster. NLL max rel err stays ~2.7e-3.
    hu = np.ascontiguousarray(hidden).view(np.uint32)
    hr = ((hu + np.uint32(1 << 21)) & np.uint32(0xFFC00000)).view(np.float32)
    h8 = hr.astype(ml_dtypes.float8_e4m3)
    # Timesteps >= len-1 never influence the output (END step has no
    # emission; Z-capture uses the pre-step state): zero them so the
    # shipped-but-padded tail of each slot compresses to nothing.
    h8[pos >= (lengths[:, None] - 1)] = 0
    wT8 = np.ascontiguousarray(
        (W * WSCALE).astype(ml_dtypes.float8_e4m3)
        .T.reshape(HC, 128, K).transpose(1, 0, 2)
    ).reshape(128, HC * K)

    in_maps = []
    gidx_all = []
    for core in range(NCORE):
        gidx = order[np.arange(BL) * NCORE + core]
        gidx_all.append(gidx)
        hpack = np.empty(
            (1, 128 * hidcol + 128 * HC * K + 4 * FLATN),
            dtype=ml_dtypes.float8_e4m3,
        )
        hid2d = hpack[0, 0 : 128 * hidcol].reshape(128, hidcol)
        for bslot, c_i, w_p, off in pieces:
            t0 = c_i * 128
            block = h8[gidx[bslot], t0 : t0 + w_p, :].reshape(w_p, HC, 128)
            hid2d[:, off : off + HC * w_p] = (
                block.transpose(2, 1, 0).reshape(128, HC * w_p)
            )
        hpack[0, 128 * hidcol : 128 * hidcol + 128 * HC * K] = wT8.reshape(-1)
        v_c = v[gidx]
        tt = np.arange(T)
        delta = (tt[:, None] == v_c[None, :]).astype(np.float32)   # [T, BL]
        cpack = np.empty((1, FLATN), dtype=np.float32)
        cpack[0, 0:DOFF] = cpack_head
        cpack[0, DOFF:] = delta.reshape(-1)
        hpack[0, 128 * hidcol + 128 * HC * K :] = cpack.reshape(-1).view(
            ml_dtypes.float8_e4m3
        )
        in_maps.append({"hpack": hpack})

    if wvals not in _NC_CACHE:
        _NC_CACHE[wvals] = build_bass(wvals)
    nc = _NC_CACHE[wvals]

    res = run_bass_kernel_spmd(nc, in_maps, core_ids=list(range(NCORE)))
    outs = res.results

    # ---- host gold score (exact f32): transitions + emissions ----
    tags_ext = np.concatenate(
        [np.full((B, 1), ROOT, tags.dtype), tags], axis=1
    )
    tr_score = (trans[tags, tags_ext[:, :-1]].astype(np.float64) * maskT).sum(axis=1)
    emit_score = np.zeros(B, dtype=np.float64)
    for core in range(NCORE):
        bs = slice(core * BL, (core + 1) * BL)
        Wg = W[tags[bs]]                                     # [BL, T, H]
        ef = np.einsum("bth,bth->bt", hidden[bs], Wg) + b[tags[bs]]
        emit_score[bs] = (ef.astype(np.float64) * emask[bs]).sum(axis=1)

    # ---- assemble nll ----
    nll = np.zeros(B, dtype=np.float64)
    ev_steps = R * np.arange(1, NEV + 1) - 1                 # [NEV]
    for core in range(NCORE):
        gidx = gidx_all[core]
        v_c = v[gidx]
        op = outs[core]["opack"][0]
        sfin = op[0 : NS * BL].reshape(NS, BL).astype(np.float64)
        scap = op[NS * BL :].reshape(NEV, BL).astype(np.float64)
        AZ = sfin[K] + sfin[K + 1]
        prefix_mask = ev_steps[:, None] < v_c[None, :]
        logS_prefix = (-np.log(scap) * prefix_mask).sum(axis=0)
        log_z = np.log(AZ) + (v_c + 1) * LOGC + logS_prefix
        nll[gidx] = log_z - tr_score[gidx] - emit_score[gidx]

    return nll.astype(np.float32)


# revision 35
# speedup vs baseline: 1.5618x; 1.0454x over previous
"""ChainCRF NLL kernel for Trainium2 (8 NeuronCores, data parallel over B).

Transfer-optimized design (the axon tunnel at ~75 MB/s dominates the span):
  - hidden ships as fp8e4m3, host-pre-transposed to [H-chunk, t] layout and
    packed with the (x16-scaled) fp8 W into one DRAM tensor per core.
  - sequences are assigned to cores round-robin by descending-length rank, so
    all cores share one static per-slot packed width wvals[b] =
    max-length-in-rank-group-b (rounded to 4); only those timesteps ship.
    Columns of the M buffer beyond a sequence's width stay at 1.0 — the
    recursion there decays geometrically and the periodic rescale
    renormalizes it, so the Z/A capture rows are unaffected.
  - gold score (transitions + emissions) is computed exactly on host in f32.
  - device computes feats via fp8 matmul, exp(feats/16 + b) into per-chunk
    M buffers, then runs the exp-domain linear recursion
        Ehat_{t+1} = expFeat_t * (TrAug @ Ehat_t)
    with TrAug carrying the exp(trans)/C block, a Z capture column (selected
    by the host-supplied delta row at t == len[b]-1), an A accumulator
    (A' = A + Z), and a 1/C ones column producing Shat for periodic rescale.
  - host: nll = [log(A+Z) + (v+1)*logC + sum of event logS before v] - gold.

The NEFF is specialized on the width tuple wvals (derived from lengths) and
cached per-process; a different length profile just triggers a recompile.
"""

import os

import numpy as np
import ml_dtypes

import jax

# Persistent XLA compilation cache: run_bass_kernel_spmd rebuilds its jit
# wrapper every call, so without this each call pays a ~0.4 s recompile.
try:
    jax.config.update(
        "jax_compilation_cache_dir", os.path.expanduser("~/.jax_comp_cache")
    )
    jax.config.update("jax_persistent_cache_min_compile_time_secs", 0.0)
    jax.config.update("jax_persistent_cache_min_entry_size_bytes", 0)
except Exception:
    pass

import concourse.bass as bass
import concourse.bacc as bacc
import concourse.tile as tile
from concourse import mybir
from concourse.bass import ds, ts
from concourse.bass_utils import run_bass_kernel_spmd

B, T, H, K = 128, 1024, 512, 52
ROOT, END = 0, 1
NCORE = 8
BL = B // NCORE          # 16 sequences per core
NS = K + 2               # state rows: 52 Ehat + Z + A
NO = 65                  # out rows: 52 U + Z + A + pad, Shat at partition 64
R = 32                   # rescale period
NEV = T // R             # 32 events
LOGC = 4.9               # constant per-step rescale (exp-domain drift removal)
WSCALE = 16.0            # fp8 range scaling for W; undone by activation scale

NCHUNK = T // 128        # 8 time chunks of 128 steps
HC = H // 128            # 4 H-chunks

F32 = mybir.dt.float32
FP8 = mybir.dt.float8e4

_NC_CACHE = {}
MBITS = 1


def _pieces(wvals):
    """Chunk-boundary pieces (b, c, w_p, off) of the exact-length packing."""
    pieces = []
    off = 0
    for b, w_b in enumerate(wvals):
        for c in range(-(-w_b // 128)):
            w_p = min(128, w_b - c * 128)
            pieces.append((b, c, w_p, off))
            off += HC * w_p
    return pieces, off


def build_bass(wvals):
    # wvals[b] = per-slot packed timestep count (multiple of 4)
    pieces, hidcol = _pieces(wvals)
    packcol = hidcol + HC * K

    nc = bacc.Bacc(None)
    # single flat fp8 input: [hid p-major | wT p-major | cpack f32 bytes]
    # cpack logical f32 layout: [trAug p-major 54*65 | bias 52 | pad 2 | delta]
    FLATN = NS * NO + K + 2 + NCHUNK * 128 * BL
    DOFF = NS * NO + K + 2
    HB = 128 * hidcol
    WB = 128 * HC * K
    TOT = HB + WB + 4 * FLATN
    hpack = nc.dram_tensor("hpack", [1, TOT], FP8, kind="ExternalInput")

    # single packed output: [sfinal p-major NS*BL | scap NEV*BL]
    opack = nc.dram_tensor(
        "opack", [1, NS * BL + NEV * BL], F32, kind="ExternalOutput"
    )

    with tile.TileContext(nc) as tc:
        with (
            tc.tile_pool(name="consts", bufs=1) as consts,
            tc.tile_pool(name="mbuf", bufs=1) as mbuf,
            tc.tile_pool(name="state", bufs=3) as spool,
            tc.tile_pool(name="pf", bufs=2, space="PSUM") as pfp,
            tc.tile_pool(name="pr", bufs=2, space="PSUM") as prpsum,
            tc.tile_pool(name="pb", bufs=1, space="PSUM") as pbp,
        ):
            # ---- constants / inputs resident in SBUF ----
            cpack = hpack[:, HB + WB : TOT].bitcast(F32)   # [1, FLATN] f32 view
            hid_sb = consts.tile([128, hidcol], FP8, tag="hid")
            nc.sync.dma_start(
                hid_sb, hpack[:, 0:HB].rearrange("a (p x) -> (a p) x", p=128)
            )
            wT_sb = consts.tile([128, HC, K], FP8, tag="wT")
            nc.sync.dma_start(
                wT_sb,
                hpack[:, HB : HB + WB].rearrange(
                    "a (p h k) -> (a p) h k", p=128, h=HC
                ),
            )
            trAug_sb = consts.tile([NS, NO], F32, tag="trAug")
            nc.sync.dma_start(
                trAug_sb,
                cpack[:, 0 : NS * NO].rearrange("a (p x) -> (a p) x", p=NS),
            )
            bias_sb = consts.tile([K, 1], F32, tag="bvec")
            nc.sync.dma_start(
                bias_sb,
                cpack[:, NS * NO : NS * NO + K].rearrange(
                    "a (p x) -> (a p) x", p=K
                ),
            )
            ones_r_sb = consts.tile([1, K], F32, tag="ones_r")
            nc.gpsimd.memset(ones_r_sb, 1.0)
            scap_sb = consts.tile([1, NEV * BL], F32, tag="scap")

            mall = mbuf.tile([NS, T * BL], F32, tag="mall")
            nc.gpsimd.memset(mall, 1.0)
            nc.sync.dma_start(
                mall[K : K + 1, :], cpack[:, DOFF : DOFF + T * BL]
            )

            s_a = spool.tile([NS, BL], F32, tag="sa")
            s_b = spool.tile([NS, BL], F32, tag="sb")
            nc.gpsimd.memset(s_a, 0.0)
            nc.gpsimd.memset(s_a[ROOT : ROOT + 1, :], 1.0)

            # ---- phase A: feats for all packed pieces ----
            for b, c, w_p, off in pieces:
                pf_t = pfp.tile([K, 128], F32, tag="pf")
                for ch in range(HC):
                    nc.tensor.matmul(
                        pf_t[:, 0:w_p],
                        wT_sb[:, ch, :],
                        hid_sb[:, off + ch * w_p : off + (ch + 1) * w_p],
                        start=(ch == 0),
                        stop=(ch == HC - 1),
                    )
                # exp(feats/WSCALE + b) into M rows 0:52 (cols strided by BL)
                mview = mall[0:K, :].rearrange(
                    "p (t b) -> p t b", b=BL
                )[:, c * 128 : c * 128 + w_p, b : b + 1]
                nc.scalar.activation(
                    mview, pf_t[:, 0:w_p], mybir.ActivationFunctionType.Exp,
                    bias=bias_sb, scale=1.0 / WSCALE,
                )

            # ---- phase B: 1024-step recursion as a hardware loop ----
            # 32 iterations x (R=32 serial steps + rescale); state ping-pongs
            # between s_a/s_b (R even => ends back in s_a each iteration).
            p0 = prpsum.tile([NO, BL], F32, tag="pr0")
            p1 = prpsum.tile([NO, BL], F32, tag="pr1")
            bc_t = pbp.tile([K, BL], F32, tag="pb")
            with tc.For_i(0, NEV) as e:
                for j in range(R):
                    src, dst = (s_a, s_b) if j % 2 == 0 else (s_b, s_a)
                    p_t = p0 if j % 2 == 0 else p1
                    nc.tensor.matmul(p_t, trAug_sb, src, start=True, stop=True)
                    nc.vector.tensor_mul(
                        dst,
                        mall[:, ds(e * (R * BL) + j * BL, BL)],
                        p_t[0:NS, :],
                    )
                srec = scap_sb[0:1, ts(e, BL)]
                nc.vector.reciprocal(srec, p1[NO - 1 : NO, :])
                nc.tensor.matmul(bc_t, ones_r_sb, srec, start=True, stop=True)
                nc.vector.tensor_mul(s_a[0:K, :], s_a[0:K, :], bc_t)

            # ---- outputs ----
            nc.sync.dma_start(
                opack[:, 0 : NS * BL].rearrange("a (p x) -> (a p) x", p=NS),
                s_a,
            )
            nc.sync.dma_start(opack[:, NS * BL :], scap_sb)

    nc.compile()
    return nc


def kernel(hidden, W, b, log_transitions, tags, lengths):
    hidden = np.asarray(hidden, dtype=np.float32)
    W = np.asarray(W, dtype=np.float32)
    b = np.asarray(b, dtype=np.float32)
    trans = np.asarray(log_transitions, dtype=np.float32)
    tags = np.asarray(tags, dtype=np.int32)
    lengths = np.asarray(lengths, dtype=np.int32)

    C = np.float64(np.exp(LOGC))
    expTr = np.exp(trans.astype(np.float64))
    trAug = np.zeros((NS, NO), dtype=np.float64)
    trAug[:K, :K] = expTr.T / C
    trAug[:K, K] = expTr[END, :] / C          # Z capture column
    trAug[K, K + 1] = 1.0                     # A' = A + Z
    trAug[K + 1, K + 1] = 1.0
    trAug[:K, NO - 1] = 1.0 / C               # Shat column (partition 64)
    trAug = trAug.astype(np.float32)

    FLATN = NS * NO + K + 2 + NCHUNK * 128 * BL
    DOFF = NS * NO + K + 2
    cpack_head = np.zeros(DOFF, dtype=np.float32)
    cpack_head[0 : NS * NO] = trAug.reshape(-1)
    cpack_head[NS * NO : NS * NO + K] = b

    # ---- length-ranked round-robin assignment + exact per-slot widths ----
    order = np.argsort(-lengths.astype(np.int64), kind="stable")
    Lsort = lengths.astype(np.int64)[order]
    wvals = tuple(
        min(T, int(-(-Lsort[bslot * NCORE] // 4)) * 4) for bslot in range(BL)
    )
    pieces, hidcol = _pieces(wvals)

    v = (lengths.astype(np.int64) - 1)        # capture step per sequence
    pos = np.arange(T)[None, :]
    maskT = pos < lengths[:, None]
    is_last = pos == (lengths[:, None] - 1)
    emask = (maskT & ~is_last)

    # ---- fp8 packed, transposed hidden ----
    # Round hidden to 1 mantissa bit (unbiased, exactly representable in
    # e4m3): the axon tunnel compresses the stream, and the lower-entropy
    # code distribution ships ~10% faster. NLL max rel err stays ~2.7e-3.
    hu = np.ascontiguousarray(hidden).view(np.uint32)
    hr = ((hu + np.uint32(1 << (22 - MBITS))) & np.uint32(
        (~((1 << (23 - MBITS)) - 1)) & 0xFFFFFFFF)).view(np.float32)
    h8 = hr.astype(ml_dtypes.float8_e4m3)
    # Timesteps >= len-1 never influence the output (END step has no
    # emission; Z-capture uses the pre-step state): zero them so the
    # shipped-but-padded tail of each slot compresses to nothing.
    h8[pos >= (lengths[:, None] - 1)] = 0
    wT8 = np.ascontiguousarray(
        (W * WSCALE).astype(ml_dtypes.float8_e4m3)
        .T.reshape(HC, 128, K).transpose(1, 0, 2)
    ).reshape(128, HC * K)

    in_maps = []
    gidx_all = []
    for core in range(NCORE):
        gidx = order[np.arange(BL) * NCORE + core]
        gidx_all.append(gidx)
        hpack = np.empty(
            (1, 128 * hidcol + 128 * HC * K + 4 * FLATN),
            dtype=ml_dtypes.float8_e4m3,
        )
        hid2d = hpack[0, 0 : 128 * hidcol].reshape(128, hidcol)
        for bslot, c_i, w_p, off in pieces:
            t0 = c_i * 128
            block = h8[gidx[bslot], t0 : t0 + w_p, :].reshape(w_p, HC, 128)
            hid2d[:, off : off + HC * w_p] = (
                block.transpose(2, 1, 0).reshape(128, HC * w_p)
            )
        hpack[0, 128 * hidcol : 128 * hidcol + 128 * HC * K] = wT8.reshape(-1)
        v_c = v[gidx]
        tt = np.arange(T)
        delta = (tt[:, None] == v_c[None, :]).astype(np.float32)   # [T, BL]
        cpack = np.empty((1, FLATN), dtype=np.float32)
        cpack[0, 0:DOFF] = cpack_head
        cpack[0, DOFF:] = delta.reshape(-1)
        hpack[0, 128 * hidcol + 128 * HC * K :] = cpack.reshape(-1).view(
            ml_dtypes.float8_e4m3
        )
        in_maps.append({"hpack": hpack})

    if wvals not in _NC_CACHE:
        _NC_CACHE[wvals] = build_bass(wvals)
    nc = _NC_CACHE[wvals]

    res = run_bass_kernel_spmd(nc, in_maps, core_ids=list(range(NCORE)))
    outs = res.results

    # ---- host gold score (exact f32): transitions + emissions ----
    tags_ext = np.concatenate(
        [np.full((B, 1), ROOT, tags.dtype), tags], axis=1
    )
    tr_score = (trans[tags, tags_ext[:, :-1]].astype(np.float64) * maskT).sum(axis=1)
    emit_score = np.zeros(B, dtype=np.float64)
    for core in range(NCORE):
        bs = slice(core * BL, (core + 1) * BL)
        Wg = W[tags[bs]]                                     # [BL, T, H]
        ef = np.einsum("bth,bth->bt", hidden[bs], Wg) + b[tags[bs]]
        emit_score[bs] = (ef.astype(np.float64) * emask[bs]).sum(axis=1)

    # ---- assemble nll ----
    nll = np.zeros(B, dtype=np.float64)
    ev_steps = R * np.arange(1, NEV + 1) - 1                 # [NEV]
    for core in range(NCORE):
        gidx = gidx_all[core]
        v_c = v[gidx]
        op = outs[core]["opack"][0]
        sfin = op[0 : NS * BL].reshape(NS, BL).astype(np.float64)
        scap = op[NS * BL :].reshape(NEV, BL).astype(np.float64)
        AZ = sfin[K] + sfin[K + 1]
        prefix_mask = ev_steps[:, None] < v_c[None, :]
        logS_prefix = (-np.log(scap) * prefix_mask).sum(axis=0)
        log_z = np.log(AZ) + (v_c + 1) * LOGC + logS_prefix
        nll[gidx] = log_z - tr_score[gidx] - emit_score[gidx]

    return nll.astype(np.float32)
